# revision 16
# baseline (speedup 1.0000x reference)
"""MultiHeadAttention (B=1, S=4096, E=1024, H=16) on 8 Trainium2 NeuronCores.

Sharding: tensor-parallel over heads. Core c computes heads 2c and 2c+1
(embed slice 128c:128c+128 of the QKV projections, column-parallel) plus its
row-slice of the output projection (row-parallel); the host sums the 8
partial outputs and adds the output bias.

Device kernel (identical SPMD program on every core, fp16 matmuls with fp32
PSUM accumulation; ~3e-4 relative error vs the fp32 reference):
  phase 1: Q^T/K^T/V^T = W_slice^T @ x^T  (x^T pre-transposed on host),
           per-partition biases fused into the PSUM->SBUF copies on DVE;
           V^T transposed on the PE into V' = [V_h0 | 1 | V_h1 | 1] whose
           ones columns produce the softmax denominators inside the attn@V
           matmul. Q is stored as two zero-padded per-head tensors so every
           scores matmul contracts the full 128 partitions (TRN2 fp16
           matmuls at mixed base partitions lock up the PE array).
  phase 2: flash attention per 512-query chunk: scores^T tile per k-chunk
           (both heads into one 2-bank PSUM tensor), exp on ScalarE with
           the 1/sqrt(dh) scale fused, attn@V' accumulated over k in PSUM;
           denominator DMA'd to partition 0, partition-broadcast on GpSimd,
           fast reciprocal + multiply on DVE; out_proj (row-parallel Wo
           slice) software-pipelined into the next chunk's k-loop, partial
           [4096, 1024] written to HBM.
"""
import numpy as np
from contextlib import ExitStack

EMBED = 1024
S = 4096
HEADS_PER_CORE = 2
DCORE = 128          # embed cols per core (2 heads x 64)
DH = 64              # head dim
NCORES = 8
EC = 8               # e-chunks of 128 (contraction for projections)
NSC = 8              # s-chunks of 512 for projections
SQW = 512            # flash query-chunk width
NSQ = S // SQW       # 8
NK = 32              # key chunks of 128
VW = 2 * (DH + 1)    # 130: V' cols per s-tile  [V_h0 | 1 | V_h1 | 1]

_CACHE = {}


def _build():
    import concourse.bacc as bacc
    import concourse.tile as tile
    from concourse import mybir

    F32 = mybir.dt.float32
    F16 = mybir.dt.float16
    AF = mybir.ActivationFunctionType

    nc = bacc.Bacc("TRN2", target_bir_lowering=False, debug=False)

    xT = nc.dram_tensor("xT", [EMBED, S], F16, kind="ExternalInput").ap()
    wq = nc.dram_tensor("wq", [EMBED, DCORE], F16, kind="ExternalInput").ap()
    wk = nc.dram_tensor("wk", [EMBED, DCORE], F16, kind="ExternalInput").ap()
    wv = nc.dram_tensor("wv", [EMBED, DCORE], F16, kind="ExternalInput").ap()
    wo = nc.dram_tensor("wo", [DCORE, EMBED], F16, kind="ExternalInput").ap()
    bq = nc.dram_tensor("bq", [DCORE, 1], F32, kind="ExternalInput").ap()
    bk = nc.dram_tensor("bk", [DCORE, 1], F32, kind="ExternalInput").ap()
    bv = nc.dram_tensor("bv", [DCORE, 1], F32, kind="ExternalInput").ap()
    ident = nc.dram_tensor("ident", [128, 128], F16, kind="ExternalInput").ap()
    ones = nc.dram_tensor("ones", [128, 2 * NK], F16, kind="ExternalInput").ap()
    out = nc.dram_tensor("out", [S, EMBED], F32, kind="ExternalOutput").ap()

    with tile.TileContext(nc) as tc, ExitStack() as ctx:
        cst = ctx.enter_context(tc.tile_pool(name="cst", bufs=1))
        big = ctx.enter_context(tc.tile_pool(name="big", bufs=1))

        # ---- constants / weights in SBUF ----
        wq_sb = cst.tile([128, EC * DCORE], F16, tag="wq")
        wk_sb = cst.tile([128, EC * DCORE], F16, tag="wk")
        wv_sb = cst.tile([128, EC * DCORE], F16, tag="wv")
        wo_sb = cst.tile([128, EMBED], F16, tag="wo")
        bq_sb = cst.tile([128, 1], F32, tag="bq")
        bk_sb = cst.tile([128, 1], F32, tag="bk")
        bv_sb = cst.tile([128, 1], F32, tag="bv")
        id_sb = cst.tile([128, 128], F16, tag="ident")

        # one DMA per weight: [e, d] -> [128, ec*d] e-chunk-major
        for w_dram, w_sb in ((wq, wq_sb), (wk, wk_sb), (wv, wv_sb)):
            nc.sync.dma_start(
                w_sb[:].rearrange("p (ec n) -> p ec n", ec=EC),
                w_dram.rearrange("(ec p) n -> p ec n", p=128),
            )
        nc.scalar.dma_start(wo_sb[:], wo)
        nc.scalar.dma_start(bq_sb[:], bq)
        nc.scalar.dma_start(bk_sb[:], bk)
        nc.scalar.dma_start(bv_sb[:], bv)
        nc.scalar.dma_start(id_sb[:], ident)

        # ---- big SBUF tensors ----
        qTp0 = big.tile([128, S], F16, tag="qTp0")  # head0 rows 0:64, rows 64:128 zero
        qTp1 = big.tile([128, S], F16, tag="qTp1")  # head1 rows 64:128, rows 0:64 zero
        kT = big.tile([128, S], F16, tag="kT")
        vT = big.tile([128, S], F16, tag="vT")
        vp = big.tile([128, NK * VW], F16, tag="vp")   # V' per 128-row s-tile
        aT = big.tile([128, S], F16, tag="aT")   # normalized attn^T

        # ones columns of V': cols 64 and 129 of each 130-wide tile
        nc.vector.memset(qTp0[64:128, :], 0.0)
        nc.vector.memset(qTp1[0:64, :], 0.0)
        ones_sb = cst.tile([128, 2 * NK], F16, tag="ones_sb")
        nc.sync.dma_start(ones_sb[:], ones)
        vp_r = vp[:].rearrange("p (t c) -> p t c", c=VW)
        nc.vector.tensor_copy(vp_r[:, :, DH::DH + 1],
                              ones_sb[:].rearrange("p (t c) -> p t c", c=2))

        # ---- phase 1: projections ----
        with (
            tc.tile_pool(name="xts", bufs=3) as xts_pool,
            tc.tile_pool(name="pps", bufs=2, space="PSUM") as pps,
            tc.tile_pool(name="vtp", bufs=2, space="PSUM") as vtps,
        ):
            for sc in range(NSC):
                xts = xts_pool.tile([128, EC * 512], F16, tag="xts")
                xts_r = xts[:].rearrange("p (ec n) -> p ec n", ec=EC)
                xT_r = xT[:, sc * 512:(sc + 1) * 512].rearrange(
                    "(ec p) n -> p ec n", p=128)
                nc.sync.dma_start(xts_r[:, 0:EC // 2], xT_r[:, 0:EC // 2])
                nc.scalar.dma_start(xts_r[:, EC // 2:EC], xT_r[:, EC // 2:EC])
                sl = slice(sc * 512, (sc + 1) * 512)
                psq = pps.tile([128, 512], F32, tag="psq")
                psk = pps.tile([128, 512], F32, tag="psk")
                psv = pps.tile([128, 512], F32, tag="psv")
                for ec in range(EC):
                    xsl = xts[:, ec * 512:(ec + 1) * 512]
                    st, sp = ec == 0, ec == EC - 1
                    nc.tensor.matmul(psq[:], wq_sb[:, ec * 128:(ec + 1) * 128], xsl,
                                     start=st, stop=sp)
                    nc.tensor.matmul(psk[:], wk_sb[:, ec * 128:(ec + 1) * 128], xsl,
                                     start=st, stop=sp)
                    nc.tensor.matmul(psv[:], wv_sb[:, ec * 128:(ec + 1) * 128], xsl,
                                     start=st, stop=sp)
                # PSUM -> SBUF with bias add (per-partition bias vectors)
                nc.vector.tensor_scalar_add(qTp0[0:64, sl], psq[0:64, :], bq_sb[0:64])
                nc.vector.tensor_scalar_add(qTp1[64:128, sl], psq[64:128, :],
                                            bq_sb[64:128])
                nc.vector.tensor_scalar_add(kT[:, sl], psk[:], bk_sb[:])
                nc.vector.tensor_scalar_add(vT[:, sl], psv[:], bv_sb[:])
                # transpose V^T s-chunk into V' tiles (PE transpose, f32r)
                for t in range(4):
                    st_idx = 4 * sc + t
                    vtp = vtps.tile([128, 128], F16, tag="vtp")
                    nc.tensor.transpose(
                        vtp[:], vT[:, st_idx * 128:(st_idx + 1) * 128], id_sb[:])
                    base = st_idx * VW
                    nc.vector.tensor_copy(vp[:, base:base + DH], vtp[:, 0:DH])
                    nc.vector.tensor_copy(
                        vp[:, base + DH + 1:base + 2 * DH + 1], vtp[:, DH:2 * DH])

        # ---- phase 2: flash attention + out_proj ----
        with (
            tc.tile_pool(name="scps", bufs=2, space="PSUM") as scps_pool,
            tc.tile_pool(name="opps", bufs=2, space="PSUM") as opps_pool,
            tc.tile_pool(name="avps", bufs=1, space="PSUM") as avps_pool,
            tc.tile_pool(name="ptp", bufs=4) as ptp,
            tc.tile_pool(name="eps", bufs=2) as eps,
            tc.tile_pool(name="osb", bufs=3) as osb_pool,
        ):
            def emit_outproj(sq):
                for t in range(SQW // 128):
                    st_idx = sq * (SQW // 128) + t
                    asl = aT[:, st_idx * 128:(st_idx + 1) * 128]
                    osb = osb_pool.tile([128, EMBED], F32, tag="osb", name="osb")
                    for half in range(2):
                        op = opps_pool.tile([128, 512], F32, tag="op", name="op")
                        nc.tensor.matmul(op[:], asl,
                                         wo_sb[:, half * 512:(half + 1) * 512],
                                         start=True, stop=True)
                        nc.vector.tensor_copy(
                            osb[:, half * 512:(half + 1) * 512], op[:])
                    nc.sync.dma_start(
                        out[st_idx * 128:(st_idx + 1) * 128, :], osb[:])

            for sq in range(NSQ):
                qsl = slice(sq * SQW, (sq + 1) * SQW)
                av0 = avps_pool.tile([65, SQW], F32, tag="av0")
                av1 = avps_pool.tile([65, SQW], F32, tag="av1")
                for k in range(NK):
                    if k == 16 and sq >= 1:
                        emit_outproj(sq - 1)
                    ksl = slice(k * 128, (k + 1) * 128)
                    scps = scps_pool.tile([128, 2 * SQW], F32, tag="sc")
                    nc.tensor.matmul(scps[:, 0:SQW], kT[:, ksl], qTp0[:, qsl],
                                     start=True, stop=True)
                    nc.tensor.matmul(scps[:, SQW:2 * SQW], kT[:, ksl], qTp1[:, qsl],
                                     start=True, stop=True)
                    pt = ptp.tile([128, 2 * SQW], F16, tag="pt")
                    nc.scalar.activation(pt[:], scps[:], AF.Exp, scale=0.125)
                    st, sp = k == 0, k == NK - 1
                    vb = k * VW
                    nc.tensor.matmul(av0[:], vp[:, vb:vb + DH + 1],
                                     pt[:, 0:SQW], start=st, stop=sp)
                    nc.tensor.matmul(av1[:], vp[:, vb + DH + 1:vb + VW],
                                     pt[:, SQW:2 * SQW], start=st, stop=sp)
                # epilogue: evacuate PSUM fast, then normalize
                for h, av in ((0, av0), (1, av1)):
                    av_sb = eps.tile([65, SQW], F32, tag=f"avsb{h}")
                    nc.vector.tensor_copy(av_sb[:], av[:])
                    den0 = eps.tile([1, SQW], F32, tag=f"den0{h}")
                    nc.sync.dma_start(den0[:], av_sb[64:65, :])
                    dbc = eps.tile([64, SQW], F32, tag=f"dbc{h}")
                    nc.gpsimd.partition_broadcast(dbc[:], den0[:])
                    rbc = eps.tile([64, SQW], F32, tag=f"rbc{h}")
                    nc.vector.reciprocal_approx_fast(rbc[:], dbc[:])
                    if h == 0:
                        nc.vector.tensor_mul(aT[0:64, qsl], av_sb[0:64, :], rbc[:])
                    else:
                        a1 = eps.tile([64, SQW], F16, tag="a1")
                        nc.vector.tensor_mul(a1[:], av_sb[0:64, :], rbc[:])
                        nc.sync.dma_start(aT[64:128, qsl], a1[:])
            emit_outproj(NSQ - 1)

    nc.compile()
    return nc


def _get_nc():
    if "nc" not in _CACHE:
        _CACHE["nc"] = _build()
    return _CACHE["nc"]


def kernel(x, Wq, bq, Wk, bk, Wv, bv, Wo, bo):
    from concourse.bass_utils import run_bass_kernel_spmd

    x = np.asarray(x, dtype=np.float32)
    xT = np.ascontiguousarray(x.reshape(S, EMBED).T.astype(np.float16))
    eye = np.eye(128, dtype=np.float16)
    in_maps = []
    for c in range(NCORES):
        sl = slice(c * DCORE, (c + 1) * DCORE)
        in_maps.append({
            "xT": xT,
            "wq": np.ascontiguousarray(np.asarray(Wq, np.float32)[:, sl].astype(np.float16)),
            "wk": np.ascontiguousarray(np.asarray(Wk, np.float32)[:, sl].astype(np.float16)),
            "wv": np.ascontiguousarray(np.asarray(Wv, np.float32)[:, sl].astype(np.float16)),
            "wo": np.ascontiguousarray(np.asarray(Wo, np.float32)[sl, :].astype(np.float16)),
            "bq": np.asarray(bq, np.float32)[sl].reshape(DCORE, 1),
            "bk": np.asarray(bk, np.float32)[sl].reshape(DCORE, 1),
            "bv": np.asarray(bv, np.float32)[sl].reshape(DCORE, 1),
            "ident": eye,
            "ones": np.ones((128, 2 * NK), dtype=np.float16),
        })
    nc = _get_nc()
    res = run_bass_kernel_spmd(nc, in_maps, core_ids=list(range(NCORES)))
    acc = np.zeros((S, EMBED), dtype=np.float64)
    for c in range(NCORES):
        acc += res.results[c]["out"]
    acc += np.asarray(bo, np.float64)
    return acc.astype(np.float32).reshape(1, S, EMBED)


# revision 17
# speedup vs baseline: 1.0112x; 1.0112x over previous
"""MultiHeadAttention (B=1, S=4096, E=1024, H=16) on 8 Trainium2 NeuronCores.

Sharding: tensor-parallel over heads. Core c computes heads 2c and 2c+1
(embed slice 128c:128c+128 of the QKV projections, column-parallel) plus its
row-slice of the output projection (row-parallel); the host sums the 8
partial outputs and adds the output bias.

Device kernel (identical SPMD program on every core, fp16 matmuls with fp32
PSUM accumulation; ~3e-4 relative error vs the fp32 reference):
  phase 1: Q^T/K^T/V^T = W_slice^T @ x^T  (x^T pre-transposed on host),
           per-partition biases fused into the PSUM->SBUF copies on DVE;
           V^T transposed on the PE into V' = [V_h0 | 1 | V_h1 | 1] whose
           ones columns produce the softmax denominators inside the attn@V
           matmul. Q is stored as two zero-padded per-head tensors so every
           scores matmul contracts the full 128 partitions (TRN2 fp16
           matmuls at mixed base partitions lock up the PE array).
  phase 2: flash attention per 512-query chunk: scores^T tile per k-chunk
           (both heads into one 2-bank PSUM tensor), exp on ScalarE with
           the 1/sqrt(dh) scale fused, attn@V' accumulated over k in PSUM;
           denominator DMA'd to partition 0, partition-broadcast on GpSimd,
           fast reciprocal + multiply on DVE; out_proj (row-parallel Wo
           slice) software-pipelined into the next chunk's k-loop, partial
           [4096, 1024] written to HBM.
"""
import numpy as np
from contextlib import ExitStack

EMBED = 1024
S = 4096
HEADS_PER_CORE = 2
DCORE = 128          # embed cols per core (2 heads x 64)
DH = 64              # head dim
NCORES = 8
EC = 8               # e-chunks of 128 (contraction for projections)
NSC = 8              # s-chunks of 512 for projections
SQW = 512            # flash query-chunk width
NSQ = S // SQW       # 8
NK = 32              # key chunks of 128
VW = 2 * (DH + 1)    # 130: V' cols per s-tile  [V_h0 | 1 | V_h1 | 1]

_CACHE = {}


def _build():
    import concourse.bacc as bacc
    import concourse.tile as tile
    from concourse import mybir

    F32 = mybir.dt.float32
    F16 = mybir.dt.float16
    AF = mybir.ActivationFunctionType

    nc = bacc.Bacc("TRN2", target_bir_lowering=False, debug=False)

    xT = nc.dram_tensor("xT", [EMBED, S], F16, kind="ExternalInput").ap()
    wq = nc.dram_tensor("wq", [EMBED, DCORE], F16, kind="ExternalInput").ap()
    wk = nc.dram_tensor("wk", [EMBED, DCORE], F16, kind="ExternalInput").ap()
    wv = nc.dram_tensor("wv", [EMBED, DCORE], F16, kind="ExternalInput").ap()
    wo = nc.dram_tensor("wo", [DCORE, EMBED], F16, kind="ExternalInput").ap()
    bq = nc.dram_tensor("bq", [DCORE, 1], F32, kind="ExternalInput").ap()
    bk = nc.dram_tensor("bk", [DCORE, 1], F32, kind="ExternalInput").ap()
    bv = nc.dram_tensor("bv", [DCORE, 1], F32, kind="ExternalInput").ap()
    ident = nc.dram_tensor("ident", [128, 128], F16, kind="ExternalInput").ap()
    ones = nc.dram_tensor("ones", [128, 2 * NK], F16, kind="ExternalInput").ap()
    out = nc.dram_tensor("out", [S, EMBED], F32, kind="ExternalOutput").ap()

    with tile.TileContext(nc) as tc, ExitStack() as ctx:
        cst = ctx.enter_context(tc.tile_pool(name="cst", bufs=1))
        big = ctx.enter_context(tc.tile_pool(name="big", bufs=1))

        # ---- constants / weights in SBUF ----
        wq_sb = cst.tile([128, EC * DCORE], F16, tag="wq")
        wk_sb = cst.tile([128, EC * DCORE], F16, tag="wk")
        wv_sb = cst.tile([128, EC * DCORE], F16, tag="wv")
        wo_sb = cst.tile([128, EMBED], F16, tag="wo")
        bq_sb = cst.tile([128, 1], F32, tag="bq")
        bk_sb = cst.tile([128, 1], F32, tag="bk")
        bv_sb = cst.tile([128, 1], F32, tag="bv")
        id_sb = cst.tile([128, 128], F16, tag="ident")

        # one DMA per weight: [e, d] -> [128, ec*d] e-chunk-major
        for w_dram, w_sb in ((wq, wq_sb), (wk, wk_sb), (wv, wv_sb)):
            nc.sync.dma_start(
                w_sb[:].rearrange("p (ec n) -> p ec n", ec=EC),
                w_dram.rearrange("(ec p) n -> p ec n", p=128),
            )
        nc.scalar.dma_start(wo_sb[:], wo)
        nc.scalar.dma_start(bq_sb[:], bq)
        nc.scalar.dma_start(bk_sb[:], bk)
        nc.scalar.dma_start(bv_sb[:], bv)
        nc.scalar.dma_start(id_sb[:], ident)

        # ---- big SBUF tensors ----
        qTp0 = big.tile([128, S], F16, tag="qTp0")  # head0 rows 0:64, rows 64:128 zero
        qTp1 = big.tile([128, S], F16, tag="qTp1")  # head1 rows 64:128, rows 0:64 zero
        kT = big.tile([128, S], F16, tag="kT")
        vT = big.tile([128, S], F16, tag="vT")
        vp = big.tile([128, NK * VW], F16, tag="vp")   # V' per 128-row s-tile
        aT = big.tile([128, S], F16, tag="aT")   # normalized attn^T

        # ones columns of V': cols 64 and 129 of each 130-wide tile
        nc.vector.memset(qTp0[64:128, :], 0.0)
        nc.vector.memset(qTp1[0:64, :], 0.0)
        ones_sb = cst.tile([128, 2 * NK], F16, tag="ones_sb")
        nc.sync.dma_start(ones_sb[:], ones)
        vp_r = vp[:].rearrange("p (t c) -> p t c", c=VW)
        nc.vector.tensor_copy(vp_r[:, :, DH::DH + 1],
                              ones_sb[:].rearrange("p (t c) -> p t c", c=2))

        # ---- phase 1: projections ----
        with (
            tc.tile_pool(name="xts", bufs=3) as xts_pool,
            tc.tile_pool(name="pps", bufs=2, space="PSUM") as pps,
            tc.tile_pool(name="vtp", bufs=2, space="PSUM") as vtps,
        ):
            for sc in range(NSC):
                xts = xts_pool.tile([128, EC * 512], F16, tag="xts")
                xts_r = xts[:].rearrange("p (ec n) -> p ec n", ec=EC)
                xT_r = xT[:, sc * 512:(sc + 1) * 512].rearrange(
                    "(ec p) n -> p ec n", p=128)
                for ec in range(EC):
                    eng = nc.sync if ec % 2 == 0 else nc.scalar
                    eng.dma_start(xts_r[:, ec:ec + 1], xT_r[:, ec:ec + 1])
                sl = slice(sc * 512, (sc + 1) * 512)
                psq = pps.tile([128, 512], F32, tag="psq")
                psk = pps.tile([128, 512], F32, tag="psk")
                psv = pps.tile([128, 512], F32, tag="psv")
                for ec in range(EC):
                    xsl = xts[:, ec * 512:(ec + 1) * 512]
                    st, sp = ec == 0, ec == EC - 1
                    nc.tensor.matmul(psq[:], wq_sb[:, ec * 128:(ec + 1) * 128], xsl,
                                     start=st, stop=sp)
                    nc.tensor.matmul(psk[:], wk_sb[:, ec * 128:(ec + 1) * 128], xsl,
                                     start=st, stop=sp)
                    nc.tensor.matmul(psv[:], wv_sb[:, ec * 128:(ec + 1) * 128], xsl,
                                     start=st, stop=sp)
                # PSUM -> SBUF with bias add (per-partition bias vectors)
                nc.vector.tensor_scalar_add(qTp0[0:64, sl], psq[0:64, :], bq_sb[0:64])
                nc.vector.tensor_scalar_add(qTp1[64:128, sl], psq[64:128, :],
                                            bq_sb[64:128])
                nc.vector.tensor_scalar_add(kT[:, sl], psk[:], bk_sb[:])
                nc.vector.tensor_scalar_add(vT[:, sl], psv[:], bv_sb[:])
                # transpose V^T s-chunk into V' tiles (PE transpose, f32r)
                for t in range(4):
                    st_idx = 4 * sc + t
                    vtp = vtps.tile([128, 128], F16, tag="vtp")
                    nc.tensor.transpose(
                        vtp[:], vT[:, st_idx * 128:(st_idx + 1) * 128], id_sb[:])
                    base = st_idx * VW
                    nc.vector.tensor_copy(vp[:, base:base + DH], vtp[:, 0:DH])
                    nc.vector.tensor_copy(
                        vp[:, base + DH + 1:base + 2 * DH + 1], vtp[:, DH:2 * DH])

        # ---- phase 2: flash attention + out_proj ----
        with (
            tc.tile_pool(name="scps", bufs=2, space="PSUM") as scps_pool,
            tc.tile_pool(name="opps", bufs=2, space="PSUM") as opps_pool,
            tc.tile_pool(name="avps", bufs=1, space="PSUM") as avps_pool,
            tc.tile_pool(name="ptp", bufs=4) as ptp,
            tc.tile_pool(name="eps", bufs=2) as eps,
            tc.tile_pool(name="osb", bufs=3) as osb_pool,
        ):
            def emit_outproj(sq):
                for t in range(SQW // 128):
                    st_idx = sq * (SQW // 128) + t
                    asl = aT[:, st_idx * 128:(st_idx + 1) * 128]
                    osb = osb_pool.tile([128, EMBED], F32, tag="osb", name="osb")
                    for half in range(2):
                        op = opps_pool.tile([128, 512], F32, tag="op", name="op")
                        nc.tensor.matmul(op[:], asl,
                                         wo_sb[:, half * 512:(half + 1) * 512],
                                         start=True, stop=True)
                        nc.vector.tensor_copy(
                            osb[:, half * 512:(half + 1) * 512], op[:])
                    nc.sync.dma_start(
                        out[st_idx * 128:(st_idx + 1) * 128, :], osb[:])

            for sq in range(NSQ):
                qsl = slice(sq * SQW, (sq + 1) * SQW)
                av0 = avps_pool.tile([65, SQW], F32, tag="av0")
                av1 = avps_pool.tile([65, SQW], F32, tag="av1")
                for k in range(NK):
                    if k == 16 and sq >= 1:
                        emit_outproj(sq - 1)
                    ksl = slice(k * 128, (k + 1) * 128)
                    scps = scps_pool.tile([128, 2 * SQW], F32, tag="sc")
                    nc.tensor.matmul(scps[:, 0:SQW], kT[:, ksl], qTp0[:, qsl],
                                     start=True, stop=True)
                    nc.tensor.matmul(scps[:, SQW:2 * SQW], kT[:, ksl], qTp1[:, qsl],
                                     start=True, stop=True)
                    pt = ptp.tile([128, 2 * SQW], F16, tag="pt")
                    nc.scalar.activation(pt[:], scps[:], AF.Exp, scale=0.125)
                    st, sp = k == 0, k == NK - 1
                    vb = k * VW
                    nc.tensor.matmul(av0[:], vp[:, vb:vb + DH + 1],
                                     pt[:, 0:SQW], start=st, stop=sp)
                    nc.tensor.matmul(av1[:], vp[:, vb + DH + 1:vb + VW],
                                     pt[:, SQW:2 * SQW], start=st, stop=sp)
                # epilogue: evacuate PSUM fast, then normalize
                for h, av in ((0, av0), (1, av1)):
                    av_sb = eps.tile([65, SQW], F32, tag=f"avsb{h}")
                    nc.vector.tensor_copy(av_sb[:], av[:])
                    den0 = eps.tile([1, SQW], F32, tag=f"den0{h}")
                    nc.sync.dma_start(den0[:], av_sb[64:65, :])
                    dbc = eps.tile([64, SQW], F32, tag=f"dbc{h}")
                    nc.gpsimd.partition_broadcast(dbc[:], den0[:])
                    rbc = eps.tile([64, SQW], F32, tag=f"rbc{h}")
                    nc.vector.reciprocal_approx_fast(rbc[:], dbc[:])
                    if h == 0:
                        nc.vector.tensor_mul(aT[0:64, qsl], av_sb[0:64, :], rbc[:])
                    else:
                        a1 = eps.tile([64, SQW], F16, tag="a1")
                        nc.vector.tensor_mul(a1[:], av_sb[0:64, :], rbc[:])
                        nc.sync.dma_start(aT[64:128, qsl], a1[:])
            emit_outproj(NSQ - 1)

    nc.compile()
    return nc


def _get_nc():
    if "nc" not in _CACHE:
        _CACHE["nc"] = _build()
    return _CACHE["nc"]


def kernel(x, Wq, bq, Wk, bk, Wv, bv, Wo, bo):
    from concourse.bass_utils import run_bass_kernel_spmd

    x = np.asarray(x, dtype=np.float32)
    xT = np.ascontiguousarray(x.reshape(S, EMBED).T.astype(np.float16))
    eye = np.eye(128, dtype=np.float16)
    in_maps = []
    for c in range(NCORES):
        sl = slice(c * DCORE, (c + 1) * DCORE)
        in_maps.append({
            "xT": xT,
            "wq": np.ascontiguousarray(np.asarray(Wq, np.float32)[:, sl].astype(np.float16)),
            "wk": np.ascontiguousarray(np.asarray(Wk, np.float32)[:, sl].astype(np.float16)),
            "wv": np.ascontiguousarray(np.asarray(Wv, np.float32)[:, sl].astype(np.float16)),
            "wo": np.ascontiguousarray(np.asarray(Wo, np.float32)[sl, :].astype(np.float16)),
            "bq": np.asarray(bq, np.float32)[sl].reshape(DCORE, 1),
            "bk": np.asarray(bk, np.float32)[sl].reshape(DCORE, 1),
            "bv": np.asarray(bv, np.float32)[sl].reshape(DCORE, 1),
            "ident": eye,
            "ones": np.ones((128, 2 * NK), dtype=np.float16),
        })
    nc = _get_nc()
    res = run_bass_kernel_spmd(nc, in_maps, core_ids=list(range(NCORES)))
    acc = np.zeros((S, EMBED), dtype=np.float64)
    for c in range(NCORES):
        acc += res.results[c]["out"]
    acc += np.asarray(bo, np.float64)
    return acc.astype(np.float32).reshape(1, S, EMBED)
